# revision 42
# baseline (speedup 1.0000x reference)
"""2-layer GAT (PyG GATConv-style) on 8 Trainium2 NeuronCores.

Sharding (dst-tile blocks): nodes padded to 160 tiles of 128; core c owns
dst-tiles [20c, 20c+20). Edges (incl. self-loops) live on the core owning
their destination, grouped by dst-tile, padded to 128-edge chunks. Dense
GEMMs are replicated (layer-1 features need no comm); per-edge src-node
feature rows are fetched with GPSIMD dma_gather from core-local HBM in
bf16. Segment softmax and scatter-aggregate run per dst-tile as one-hot
matmuls; the one-hot matrices (s01 scatter / t01 expand) are built
on-device with vector is_equal from tiny per-edge dst-local index vectors
(t01 via a partition-broadcast DMA of the same vector). H columns are
head-interleaved (col = c*8+h) and each row carries a block of ones so the
alpha-weighting of messages plus the softmax-denominator append is ONE
contiguous vector multiply. Between layers two AllGathers (split for
overlap) exchange per-node [h2@W2 | 1 | asrc2 | adst2] bf16 rows so
layer-2 gathers can read any source node.
"""

import os

import numpy as np
import ml_dtypes

# a crashed prior run can leave the NeuronCores downclocked ~1.2x;
# requesting a core reset at session init restores nominal clocks
os.environ.setdefault("NEURON_RT_RESET_CORES", "1")

import concourse.bass as bass
import concourse.mybir as mybir
import concourse.tile as tile
from concourse import bacc
from concourse.bass_utils import run_bass_kernel_spmd

dt = mybir.dt
bf16 = ml_dtypes.bfloat16

N_CORES = 8
N_NODES = 20000
IN_CH = 128
HID = 32
HEADS = 8
HC = HEADS * HID  # 256
OUT_CH = 64
NEG_SLOPE = 0.2

P = 128
N_TILES_TOTAL = 157  # ceil(20000/128)
TILES_PER_CORE = 20  # 8*20 = 160 >= 157
N_PAD = 160 * P      # 20480
LOC_NODES = TILES_PER_CORE * P  # 2560
HALF = TILES_PER_CORE // 2

ROW1 = 384  # Hbuf row (bf16): [H'(256) | ones(8) | asrc(8) | adst(8) | pad]
ROW2 = 128  # h2 row (bf16):   [hW2(64) | one | asrc2 | adst2 | pad]

AF = mybir.ActivationFunctionType
OP = mybir.AluOpType

LAST_RESULTS = None


def _prep_edges(edge_index):
    src = np.asarray(edge_index[0], dtype=np.int64)
    dst = np.asarray(edge_index[1], dtype=np.int64)
    loops = np.arange(N_NODES, dtype=np.int64)
    src = np.concatenate([src, loops])
    dst = np.concatenate([dst, loops])

    order = np.lexsort((src, dst))
    src, dst = src[order], dst[order]
    tile_of = dst // P
    core_of = np.minimum(tile_of // TILES_PER_CORE, N_CORES - 1)

    per = [[None] * TILES_PER_CORE for _ in range(N_CORES)]
    for c in range(N_CORES):
        mc = core_of == c
        sc, tc_, dc = src[mc], tile_of[mc], dst[mc]
        for j in range(TILES_PER_CORE):
            gt = c * TILES_PER_CORE + j
            mt = tc_ == gt
            per[c][j] = (sc[mt], dc[mt] - gt * P)

    G = [max(max(1, -(-len(per[c][j][0]) // P)) for c in range(N_CORES))
         for j in range(TILES_PER_CORE)]

    # phase-D edge lists split by src half (h2allA/h2allB row spaces):
    # src g -> core c2=g//2560, local l=g%2560, half=l//1280,
    # row in h2all{A,B} = c2*1280 + l%1280
    perD = [[None] * TILES_PER_CORE for _ in range(N_CORES)]
    for c in range(N_CORES):
        for j in range(TILES_PER_CORE):
            s, dl = per[c][j]
            c2 = s // LOC_NODES
            l = s % LOC_NODES
            hf = l // (HALF * P)
            row = c2 * (HALF * P) + l % (HALF * P)
            perD[c][j] = ((row[hf == 0], dl[hf == 0]),
                          (row[hf == 1], dl[hf == 1]))
    GD = [[max(max(1, -(-len(perD[c][j][h][0]) // P)) for c in range(N_CORES))
           for j in range(TILES_PER_CORE)] for h in range(2)]

    def pack(lists, Gtab):
        """lists[j] = (idx_array, dl_array); returns idx16, dlc, dlr."""
        idx_cols, dl_lin = [], []
        for j in range(TILES_PER_CORE):
            s, dl = lists[j]
            n_pad = Gtab[j] * P
            sp = np.zeros(n_pad, dtype=np.int64)
            sp[: len(s)] = s
            dlp = np.full(n_pad, 200, dtype=np.int64)
            dlp[: len(dl)] = dl
            # dma_gather: idx k -> partition k%128, chunk k//128
            idx16 = sp.astype(np.int16).reshape(n_pad // 16, 16).T
            idx_cols.append(np.tile(idx16, (8, 1)))
            dl_lin.append(dlp)
        dl_lin = np.concatenate(dl_lin)
        ncht = len(dl_lin) // P
        # dlr rides a 128-partition broadcast DMA -> int8 to halve the bytes
        # (pad 200 wraps to -56: still never equal to a partition index)
        return (np.ascontiguousarray(np.concatenate(idx_cols, axis=1)),
                np.ascontiguousarray(dl_lin.reshape(ncht, P).T).astype(bf16),
                np.ascontiguousarray(dl_lin[None, :]).astype(np.int8))

    # Hbuf rows are stored partition-major (row = (g%128)*160 + g//128) so
    # phase A's staged writes are one contiguous descriptor per partition;
    # the gather just uses remapped indices
    perH = [[((s % P) * 160 + s // P, dl) for (s, dl) in per[c]]
            for c in range(N_CORES)]
    meta = []
    for c in range(N_CORES):
        idx, dlc, dlr = pack(perH[c], G)
        i0, c0, r0 = pack([perD[c][j][0] for j in range(TILES_PER_CORE)], GD[0])
        i1, c1, r1 = pack([perD[c][j][1] for j in range(TILES_PER_CORE)], GD[1])
        meta.append({"idx": idx, "dlc": dlc, "dlr": dlr,
                     "idx0": i0, "dlc0": c0, "dlr0": r0,
                     "idx1": i1, "dlc1": c1, "dlr1": r1})
    return G, GD, meta


def _build_program(G, GD):
    NCH = sum(G)
    NCH0 = sum(GD[0])
    NCH1 = sum(GD[1])
    nc = bacc.Bacc(None, target_bir_lowering=False, debug=False,
                   num_swdge_queues=4)

    xT = nc.dram_tensor("xT", [P, N_PAD], dt.bfloat16, kind="ExternalInput")
    xTloc = nc.dram_tensor("xTloc", [P, LOC_NODES], dt.bfloat16, kind="ExternalInput")
    W1a = nc.dram_tensor("W1a", [P, HC + 32], dt.bfloat16, kind="ExternalInput")
    W2a = nc.dram_tensor("W2a", [HC, OUT_CH + 2], dt.bfloat16, kind="ExternalInput")
    csT = nc.dram_tensor("cs", [P, OUT_CH + 2], dt.float32, kind="ExternalInput")
    b1r = nc.dram_tensor("b1r", [P, HC], dt.float32, kind="ExternalInput")
    b2r = nc.dram_tensor("b2r", [P, OUT_CH], dt.float32, kind="ExternalInput")
    idxT = nc.dram_tensor("idx", [P, 8 * NCH], dt.int16, kind="ExternalInput")
    dlcT = nc.dram_tensor("dlc", [P, NCH], dt.bfloat16, kind="ExternalInput")
    idx0T = nc.dram_tensor("idx0", [P, 8 * NCH0], dt.int16, kind="ExternalInput")
    dlc0T = nc.dram_tensor("dlc0", [P, NCH0], dt.bfloat16, kind="ExternalInput")
    idx1T = nc.dram_tensor("idx1", [P, 8 * NCH1], dt.int16, kind="ExternalInput")
    dlc1T = nc.dram_tensor("dlc1", [P, NCH1], dt.bfloat16, kind="ExternalInput")
    outT = nc.dram_tensor("out", [LOC_NODES, OUT_CH], dt.float32, kind="ExternalOutput")

    GRP = 8
    Gmax = max(G)
    Gmax2 = max(max(GD[0]), max(GD[1]))
    # gathers ride SWDGE queues 1-3: queue k's descgen runs on Q7 core pair
    # k and (unlike queue 0) retires on the Pool engine in ~100ns, so three
    # pairs generate descriptors concurrently while the engine stays free
    qstate = [0]

    def qn():
        qstate[0] = qstate[0] % 3 + 1
        return qstate[0]

    with tile.TileContext(nc) as tc:
        with (
            tc.tile_pool(name="const", bufs=1) as cp,
            tc.tile_pool(name="stage", bufs=2) as stp,
            tc.tile_pool(name="stageA", bufs=3) as stpA,
            tc.tile_pool(name="edgeg", bufs=5) as epg,
            tc.tile_pool(name="edges", bufs=2) as eps,
            tc.tile_pool(name="psA", bufs=1, space="PSUM") as ppA,
            tc.tile_pool(name="psB", bufs=2, space="PSUM") as ppB,
            tc.tile_pool(name="psC", bufs=1, space="PSUM") as ppC,
            tc.tile_pool(name="psD", bufs=1, space="PSUM") as ppD,
            tc.tile_pool(name="dram", bufs=1, space="DRAM") as dp,
        ):
            # ---- constants ----
            w1a_sb = cp.tile([P, HC + 32], dt.bfloat16)
            nc.sync.dma_start(w1a_sb[:], W1a[:, :])
            w2a_sb = cp.tile([P, 2, OUT_CH + 2], dt.bfloat16)
            nc.sync.dma_start(w2a_sb[:],
                              W2a[:, :].rearrange("(k p) n -> p k n", p=P))
            cs_sb = cp.tile([P, OUT_CH + 2], dt.float32)
            nc.sync.dma_start(cs_sb[:], csT[:, :])
            b1_sb = cp.tile([P, HC], dt.float32)
            nc.sync.dma_start(b1_sb[:], b1r[:, :])
            b2_sb = cp.tile([P, OUT_CH], dt.float32)
            nc.sync.dma_start(b2_sb[:], b2r[:, :])
            idx_sb = cp.tile([P, 8 * NCH], dt.int16)
            nc.sync.dma_start(idx_sb[:], idxT[:, :])
            dlc_sb = cp.tile([P, -(-NCH // P) * P], dt.bfloat16)
            nc.sync.dma_start(dlc_sb[:, 0:NCH], dlcT[:, :])
            idx0_sb = cp.tile([P, 8 * NCH0], dt.int16)
            nc.sync.dma_start(idx0_sb[:], idx0T[:, :])
            dlc0_sb = cp.tile([P, -(-NCH0 // P) * P], dt.bfloat16)
            nc.sync.dma_start(dlc0_sb[:, 0:NCH0], dlc0T[:, :])
            idx1_sb = cp.tile([P, 8 * NCH1], dt.int16)
            nc.sync.dma_start(idx1_sb[:], idx1T[:, :])
            dlc1_sb = cp.tile([P, -(-NCH1 // P) * P], dt.bfloat16)
            nc.sync.dma_start(dlc1_sb[:, 0:NCH1], dlc1T[:, :])

            iot_row = cp.tile([P, P], dt.float32)
            nc.gpsimd.iota(iot_row[:], pattern=[[1, P]], base=0,
                           channel_multiplier=0,
                           allow_small_or_imprecise_dtypes=True)
            iot_col = cp.tile([P, 1], dt.float32)
            nc.gpsimd.iota(iot_col[:], pattern=[[0, 1]], base=0,
                           channel_multiplier=1,
                           allow_small_or_imprecise_dtypes=True)
            iot_row_bf = cp.tile([P, P], dt.bfloat16)
            nc.vector.tensor_copy(iot_row_bf[:], iot_row[:])
            iot_col_bf = cp.tile([P, 1], dt.bfloat16)
            nc.vector.tensor_copy(iot_col_bf[:], iot_col[:])
            ident = cp.tile([P, P], dt.bfloat16)
            nc.vector.tensor_scalar(ident[:], iot_row[:], iot_col[:], None,
                                    op0=OP.is_equal)
            zeros_sb = cp.tile([P, HC], dt.float32)
            nc.vector.memset(zeros_sb[:], 0.0)
            ones_sb = cp.tile([P, P], dt.bfloat16)
            nc.vector.memset(ones_sb[:], 1.0)

            Hbuf = dp.tile([N_PAD, ROW1], dt.bfloat16)
            h2rows = dp.tile([LOC_NODES, ROW2], dt.bfloat16)
            h2allA = dp.tile([N_CORES * HALF * P, ROW2], dt.bfloat16,
                             addr_space="Shared")
            h2allB = dp.tile([N_CORES * HALF * P, ROW2], dt.bfloat16,
                             addr_space="Shared")

            # ---- phase A: GEMM1 for all node tiles -> Hbuf rows ----
            # Hbuf row: [H'(256) | zeros(8) | asrc(8) | adst(8) | pad];
            # rows partition-major so each partition's group-write is one
            # contiguous descriptor (cols 288:384 are never-read garbage)
            hview = Hbuf[:, :].rearrange("(p m) r -> p m r", m=160)
            for grp in range((N_TILES_TOTAL + GRP - 1) // GRP):
                n_in = min(GRP, N_TILES_TOTAL - grp * GRP)
                stg = stpA.tile([P, GRP, ROW1], dt.bfloat16, tag="stgA")
                xch = stpA.tile([P, GRP * P], dt.bfloat16, tag="xch")
                nc.sync.dma_start(xch[:, 0:n_in * P],
                                  xT[:, grp * GRP * P:(grp * GRP + n_in) * P])
                for k in range(n_in):
                    pool = ppA if k % 2 == 0 else ppB
                    tag = "psA" if k % 2 == 0 else "ps_o"
                    psA = pool.tile([P, HC + 32], dt.float32, tag=tag)
                    nc.tensor.matmul(psA[:],
                                     xch[:, k * P:(k + 1) * P],
                                     w1a_sb[:], start=True, stop=True)
                    if k % 2 == 0:
                        nc.scalar.copy(stg[:, k, 0:HC + 32], psA[:])
                    else:
                        nc.vector.tensor_copy(stg[:, k, 0:HC + 32], psA[:])
                nc.sync.dma_start(
                    hview[:, grp * GRP:grp * GRP + n_in, :],
                    stg[:, 0:n_in, :])

            # ---- local [asrc|adst] for this core's dst tiles ----
            xl_sb = cp.tile([P, LOC_NODES], dt.bfloat16)
            nc.sync.dma_start(xl_sb[:], xTloc[:, :])
            loc_a = cp.tile([P, TILES_PER_CORE, 16], dt.bfloat16)
            for jh in range(TILES_PER_CORE):
                psL = ppA.tile([P, HC + 32], dt.float32, tag="psA")
                nc.tensor.matmul(psL[:, 0:16],
                                 xl_sb[:, jh * P:(jh + 1) * P],
                                 w1a_sb[:, HC + 8:HC + 24], start=True, stop=True)
                nc.vector.tensor_copy(loc_a[:, jh:jh + 1, :],
                                      psL[:, 0:16].unsqueeze(1))

            loc_adst2 = cp.tile([P, TILES_PER_CORE, 1], dt.bfloat16)
            coff = [0]
            for j in range(1, TILES_PER_CORE + 1):
                coff.append(coff[-1] + G[j - 1])

            def build_onehots(Gj, co, dlc_t):
                """s01[p,g,m] = (dl[p+128g]==m); t01[p,g,m] = (dl[m+128g]==p)."""
                # partition-broadcast of each chunk's dst-local vector via a
                # PE transpose of the broadcast dlc column (no DMA: a
                # broadcast DMA would wait on a cumulative HWDGE-lane sem
                # that entangles it with per-tile output writes, ~19us)
                # t01's is_equal reads ps_dlb from PSUM: the PSUM read port
                # is NOT the POOL-shared SBUF port, so the op is immune to
                # Q7 descgen port starvation (SBUF-sourced 4x is_equal
                # degrades 10-24x while gather descriptors are generated)
                t01 = eps.tile([P, Gj, P], dt.bfloat16, tag="t01",
                               padded_shape=[P, Gmax, P])
                for g0 in range(0, Gj, 8):
                    g1 = min(g0 + 8, Gj)
                    ps_dlb = ppD.tile([P, 8, P], dt.bfloat16, tag="ps_dlb")
                    for g in range(g0, g1):
                        nc.tensor.transpose(
                            ps_dlb[:, g - g0, :],
                            dlc_t[:, co + g:co + g + 1].to_broadcast([P, P]),
                            ident[:])
                    nc.vector.tensor_scalar(t01[:, g0:g1, :],
                                            ps_dlb[:, 0:g1 - g0, :],
                                            iot_col[:], None, op0=OP.is_equal)
                s01 = eps.tile([P, Gj, P], dt.bfloat16, tag="s01",
                               padded_shape=[P, Gmax, P])
                nc.vector.tensor_tensor(
                    s01[:],
                    iot_row_bf[:].unsqueeze(1).broadcast_to([P, Gj, P]),
                    dlc_t[:, co:co + Gj].unsqueeze(2)
                    .broadcast_to([P, Gj, P]),
                    op=OP.is_equal)
                return s01, t01

            # ---- phase B: layer-1 edges ----
            for j in range(TILES_PER_CORE):
                Gj = G[j]
                ni = Gj * P
                hg = epg.tile([P, Gj, ROW1], dt.bfloat16, tag="hg",
                              padded_shape=[P, Gmax, ROW1])
                nc.gpsimd.dma_gather(hg[:, 0:Gj, :], Hbuf[:, :],
                                     idx_sb[:, 8 * coff[j]:8 * (coff[j] + Gj)],
                                     num_idxs=ni, num_idxs_reg=ni,
                                     elem_size=ROW1, single_packet=False,
                                     queue_num=qn())
                s01, t01 = build_onehots(Gj, coff[j], dlc_sb)

                ps_ad = ppB.tile([P, Gmax, 8], dt.float32, tag="ps_ad")
                for g in range(Gj):
                    nc.tensor.matmul(ps_ad[:, g, :], t01[:, g, :],
                                     loc_a[:, j, 8:16], start=True, stop=True)
                er = stp.tile([P, Gj, 8], dt.float32, tag="er",
                              padded_shape=[P, Gmax, 8])
                nc.vector.tensor_tensor(er[:], hg[:, 0:Gj, HC + 8:HC + 16],
                                        ps_ad[:, 0:Gj, :], op=OP.add)
                er2 = stp.tile([P, Gj, 8], dt.float32, tag="er2",
                               padded_shape=[P, Gmax, 8])
                nc.vector.scalar_tensor_tensor(er2[:], er[:], NEG_SLOPE, er[:],
                                               op0=OP.mult, op1=OP.max)
                # alpha = exp(er2) written by ACT straight into the
                # denominator columns of mg; the multiply then only covers
                # the 256 H' columns (vs 384 before)
                alp = stp.tile([P, Gj, 8], dt.bfloat16, tag="alp",
                               padded_shape=[P, Gmax, 8])
                nc.scalar.activation(alp[:], er2[:], AF.Exp)
                mg = eps.tile([P, Gj, HC + 8], dt.bfloat16, tag="mg",
                              padded_shape=[P, Gmax, HC + 8])
                nc.vector.tensor_copy(mg[:, 0:Gj, HC:HC + 8], alp[:])
                nc.vector.tensor_tensor(
                    mg[:, 0:Gj, 0:HC].rearrange("p g (c h) -> p g c h",
                                                h=HEADS),
                    hg[:, 0:Gj, 0:HC].rearrange("p g (c h) -> p g c h",
                                                h=HEADS),
                    alp[:].unsqueeze(2).broadcast_to([P, Gj, HID, 8]),
                    op=OP.mult)

                ps_of = ppB.tile([P, HC + 32], dt.float32, tag="ps_o")
                ps_o = ps_of[:, 0:HC + 8]
                for g in range(Gj):
                    nc.tensor.matmul(ps_o[:], s01[:, g, :],
                                     mg[:, g, 0:HC + 8],
                                     start=(g == 0), stop=(g == Gj - 1))

                den = stp.tile([P, 8], dt.float32, tag="den")
                nc.vector.tensor_scalar(den[:], ps_o[:, HC:HC + 8], 1e-16,
                                        None, op0=OP.add)
                rec = stp.tile([P, 8], dt.float32, tag="rec")
                nc.vector.reciprocal(rec[:], den[:])
                o1 = stp.tile([P, HC], dt.float32, tag="o1")
                nc.vector.tensor_tensor(
                    o1[:].rearrange("p (c h) -> p c h", h=HEADS),
                    ps_o[:, 0:HC].rearrange("p (c h) -> p c h", h=HEADS),
                    rec[:].unsqueeze(1).broadcast_to([P, HID, 8]),
                    op=OP.mult)
                o1b = stp.tile([P, HC], dt.float32, tag="o1b")
                nc.vector.tensor_tensor(o1b[:], o1[:], b1_sb[:], op=OP.add)
                # ELU+1 (the -1 is folded into the cs correction after GEMM2)
                # min via tensor_tensor against zeros: tensor_scalar MIN runs
                # on a pathologically slow uop (~5.9us vs ~0.3us)
                en = stp.tile([P, HC], dt.float32, tag="en")
                nc.vector.tensor_tensor(en[:], o1b[:], zeros_sb[:], op=OP.min)
                ex = stp.tile([P, HC], dt.float32, tag="ex")
                nc.scalar.activation(ex[:], en[:], AF.Exp)
                h2p = stp.tile([P, HC], dt.bfloat16, tag="h2p")
                nc.vector.scalar_tensor_tensor(h2p[:], o1b[:], 0.0, ex[:],
                                               op0=OP.max, op1=OP.add)

                h2T = stp.tile([P, 2, P], dt.bfloat16, tag="h2T")
                for k in range(2):
                    pst = ppC.tile([P, P], dt.bfloat16, tag="pst")
                    nc.tensor.transpose(pst[:], h2p[:, k * P:(k + 1) * P],
                                        ident[:])
                    nc.vector.tensor_copy(h2T[:, k, :], pst[:])
                ps2 = ppC.tile([P, OUT_CH + 2], dt.float32, tag="ps2")
                for k in range(2):
                    nc.tensor.matmul(ps2[:], h2T[:, k, :], w2a_sb[:, k, :],
                                     start=(k == 0), stop=(k == 1))
                # row2: [hW2(64) | one | asrc2 | adst2]; (h2p-1)@W2a = ps2-cs
                row2 = stp.tile([P, ROW2], dt.bfloat16, tag="row2")
                nc.vector.tensor_tensor(row2[:, 0:OUT_CH], ps2[:, 0:OUT_CH],
                                        cs_sb[:, 0:OUT_CH], op=OP.subtract)
                nc.vector.tensor_tensor(row2[:, OUT_CH + 1:OUT_CH + 3],
                                        ps2[:, OUT_CH:OUT_CH + 2],
                                        cs_sb[:, OUT_CH:OUT_CH + 2],
                                        op=OP.subtract)
                nc.vector.tensor_copy(loc_adst2[:, j, :],
                                      row2[:, OUT_CH + 2:OUT_CH + 3])
                nc.sync.dma_start(
                    h2rows[:, :].rearrange("(t p) r -> p t r", p=P)
                    [:, j, 0:OUT_CH + 3],
                    row2[:, 0:OUT_CH + 3])

                # first-half allgather as soon as tiles 0..HALF-1 are done
                if j == HALF - 1:
                    nc.gpsimd.collective_compute(
                        "AllGather", OP.bypass,
                        replica_groups=[list(range(N_CORES))],
                        ins=[h2rows[0:HALF * P, :].opt()],
                        outs=[h2allA[:, :].opt()])

            # ---- phase C: second-half allgather ----
            nc.gpsimd.collective_compute(
                "AllGather", OP.bypass,
                replica_groups=[list(range(N_CORES))],
                ins=[h2rows[HALF * P:LOC_NODES, :].opt()],
                outs=[h2allB[:, :].opt()])

            # ---- phase D: layer-2 edges, split by src half ----
            # half-0 gathers/compute depend only on h2allA (ready during
            # phase B), so they fill the AG2 bubble; partial sums park in
            # SBUF until half-1.
            coff0 = [0]
            for j in range(1, TILES_PER_CORE + 1):
                coff0.append(coff0[-1] + GD[0][j - 1])
            coff1 = [0]
            for j in range(1, TILES_PER_CORE + 1):
                coff1.append(coff1[-1] + GD[1][j - 1])
            part3 = cp.tile([P, TILES_PER_CORE, OUT_CH + 1], dt.float32)

            def phase_d_half(j, Gj, co, h2src, idx_t, dlc_t):
                ni = Gj * P
                hg2 = epg.tile([P, Gj, ROW2], dt.bfloat16, tag="hg2",
                               padded_shape=[P, Gmax2, ROW2])
                nc.gpsimd.dma_gather(hg2[:, 0:Gj, :], h2src[:, :],
                                     idx_t[:, 8 * co:8 * (co + Gj)],
                                     num_idxs=ni, num_idxs_reg=ni,
                                     elem_size=ROW2, single_packet=False,
                                     queue_num=qn())
                s01, t01 = build_onehots(Gj, co, dlc_t)

                ps_a2f = ppB.tile([P, Gmax, 8], dt.float32, tag="ps_ad")
                ps_a2 = ps_a2f[:, :, 0:1]
                for g in range(Gj):
                    nc.tensor.matmul(ps_a2[:, g, :], t01[:, g, :],
                                     loc_adst2[:, j, :], start=True, stop=True)
                e2 = stp.tile([P, Gj, 1], dt.float32, tag="e2",
                              padded_shape=[P, Gmax2, 1])
                nc.vector.tensor_tensor(e2[:],
                                        hg2[:, 0:Gj, OUT_CH + 1:OUT_CH + 2],
                                        ps_a2[:, 0:Gj, :], op=OP.add)
                e2b = stp.tile([P, Gj, 1], dt.float32, tag="e2b",
                               padded_shape=[P, Gmax2, 1])
                nc.vector.scalar_tensor_tensor(e2b[:], e2[:], NEG_SLOPE, e2[:],
                                               op0=OP.mult, op1=OP.max)
                # alpha2 into the denominator column (64) directly; multiply
                # covers only the 64 hW2 columns (vs 128 before)
                al2 = stp.tile([P, Gj, 1], dt.bfloat16, tag="al2",
                               padded_shape=[P, Gmax2, 1])
                nc.scalar.activation(al2[:], e2b[:], AF.Exp)
                mg2 = eps.tile([P, Gj, OUT_CH + 1], dt.bfloat16, tag="mg2",
                               padded_shape=[P, Gmax2, OUT_CH + 1])
                nc.vector.tensor_copy(mg2[:, 0:Gj, OUT_CH:OUT_CH + 1], al2[:])
                nc.vector.tensor_tensor(
                    mg2[:, 0:Gj, 0:OUT_CH], hg2[:, 0:Gj, 0:OUT_CH],
                    al2[:].broadcast_to([P, Gj, OUT_CH]),
                    op=OP.mult)

                ps3f = ppC.tile([P, OUT_CH + 2], dt.float32, tag="ps2")
                ps3 = ps3f[:, 0:OUT_CH + 1]
                for g in range(Gj):
                    nc.tensor.matmul(ps3[:], s01[:, g, :],
                                     mg2[:, g, 0:OUT_CH + 1],
                                     start=(g == 0), stop=(g == Gj - 1))
                return ps3

            for j in range(TILES_PER_CORE):
                ps3 = phase_d_half(j, GD[0][j], coff0[j], h2allA,
                                   idx0_sb, dlc0_sb)
                nc.vector.tensor_copy(part3[:, j, :], ps3[:])

            for j in range(TILES_PER_CORE):
                ps3 = phase_d_half(j, GD[1][j], coff1[j], h2allB,
                                   idx1_sb, dlc1_sb)
                o3s = stp.tile([P, OUT_CH + 1], dt.float32, tag="o3s")
                nc.vector.tensor_tensor(o3s[:], ps3[:], part3[:, j, :],
                                        op=OP.add)
                den2 = stp.tile([P, 1], dt.float32, tag="den2")
                nc.vector.tensor_scalar(den2[:], o3s[:, OUT_CH:OUT_CH + 1],
                                        1e-16, None, op0=OP.add)
                rec2 = stp.tile([P, 1], dt.float32, tag="rec2")
                nc.vector.reciprocal(rec2[:], den2[:])
                o2 = stp.tile([P, OUT_CH], dt.float32, tag="o2")
                nc.vector.tensor_tensor(o2[:], o3s[:, 0:OUT_CH],
                                        rec2[:].broadcast_to([P, OUT_CH]),
                                        op=OP.mult)
                o2b = stp.tile([P, OUT_CH], dt.float32, tag="o2b")
                nc.vector.tensor_tensor(o2b[:], o2[:], b2_sb[:], op=OP.add)
                nc.sync.dma_start(
                    outT[:, :].rearrange("(t p) r -> p t r", p=P)[:, j, :],
                    o2b[:])

    nc.compile()
    return nc


def kernel(x, edge_index, W1, a_src1, a_dst1, b1, W2, a_src2, a_dst2, b2,
           _trace=False, _tmpdir=None):
    x = np.asarray(x, dtype=np.float32)
    W1 = np.asarray(W1, dtype=np.float32)
    a_src1 = np.asarray(a_src1, dtype=np.float32)
    a_dst1 = np.asarray(a_dst1, dtype=np.float32)
    b1 = np.asarray(b1, dtype=np.float32)
    W2 = np.asarray(W2, dtype=np.float32)
    a_src2 = np.asarray(a_src2, dtype=np.float32)
    a_dst2 = np.asarray(a_dst2, dtype=np.float32)
    b2 = np.asarray(b2, dtype=np.float32)

    G, GD, meta = _prep_edges(edge_index)

    A1 = np.zeros((HC, 16), np.float32)
    for h in range(HEADS):
        A1[h * HID:(h + 1) * HID, h] = a_src1[h]
        A1[h * HID:(h + 1) * HID, 8 + h] = a_dst1[h]
    # W1' columns head-interleaved: col c*8+h = W1 col h*32+c
    W1i = np.ascontiguousarray(
        W1.reshape(IN_CH, HEADS, HID).transpose(0, 2, 1).reshape(IN_CH, HC))
    W1a = np.ascontiguousarray(
        np.concatenate([W1i, np.zeros((IN_CH, 8), np.float32), W1 @ A1,
                        np.zeros((IN_CH, 8), np.float32)],
                       axis=1)).astype(bf16)
    # W2 rows permuted to match interleaved h2 (row c*8+h = W2 row h*32+c)
    W2i = np.ascontiguousarray(
        W2.reshape(HEADS, HID, OUT_CH).transpose(1, 0, 2).reshape(HC, OUT_CH))
    a_src2i = a_src2  # [1, 64] acts on output cols, not affected
    W2af = np.concatenate([W2i, W2i @ a_src2.T, W2i @ a_dst2.T], axis=1)
    W2a = np.ascontiguousarray(W2af).astype(bf16)
    # colsum correction (h2 = h2p - 1): subtract colsum(W2a) after GEMM2
    cs = np.ascontiguousarray(
        np.tile(W2af.astype(np.float32).sum(axis=0)[None, :], (P, 1)))
    b1i = b1.reshape(HEADS, HID).T.reshape(HC)

    xT = np.zeros((P, N_PAD), np.float32)
    xT[:, :N_NODES] = x.T
    xT = xT.astype(bf16)
    b1r = np.ascontiguousarray(np.tile(b1i[None, :], (P, 1)).astype(np.float32))
    b2r = np.ascontiguousarray(np.tile(b2[None, :], (P, 1)).astype(np.float32))

    nc = _build_program(G, GD)

    in_maps = []
    for c in range(N_CORES):
        in_maps.append({
            "xT": xT,
            "xTloc": np.ascontiguousarray(
                xT[:, c * LOC_NODES:(c + 1) * LOC_NODES]),
            "W1a": W1a, "W2a": W2a, "cs": cs, "b1r": b1r, "b2r": b2r,
            "idx": meta[c]["idx"], "dlc": meta[c]["dlc"],
            "idx0": meta[c]["idx0"], "dlc0": meta[c]["dlc0"],
            "idx1": meta[c]["idx1"], "dlc1": meta[c]["dlc1"],
        })

    res = run_bass_kernel_spmd(nc, in_maps, core_ids=list(range(N_CORES)),
                               trace=_trace, tmpdir=_tmpdir)
    global LAST_RESULTS
    LAST_RESULTS = res
    out = np.concatenate([res.results[c]["out"] for c in range(N_CORES)], axis=0)
    return out[:N_NODES]



# revision 44
# speedup vs baseline: 1.3023x; 1.3023x over previous
"""2-layer GAT (PyG GATConv-style) on 8 Trainium2 NeuronCores.

Sharding (dst-tile blocks): nodes padded to 160 tiles of 128; core c owns
dst-tiles [20c, 20c+20). Edges (incl. self-loops) live on the core owning
their destination, grouped by dst-tile, padded to 128-edge chunks. Dense
GEMMs are replicated (layer-1 features need no comm); per-edge src-node
feature rows are fetched with GPSIMD dma_gather from core-local HBM in
bf16. Segment softmax and scatter-aggregate run per dst-tile as one-hot
matmuls; the one-hot matrices (s01 scatter / t01 expand) are built
on-device with vector is_equal from tiny per-edge dst-local index vectors
(t01 via a partition-broadcast DMA of the same vector). H columns are
head-interleaved (col = c*8+h) and each row carries a block of ones so the
alpha-weighting of messages plus the softmax-denominator append is ONE
contiguous vector multiply. Between layers two AllGathers (split for
overlap) exchange per-node [h2@W2 | 1 | asrc2 | adst2] bf16 rows so
layer-2 gathers can read any source node.
"""

import os

import numpy as np
import ml_dtypes

# a crashed prior run can leave the NeuronCores downclocked ~1.2x;
# requesting a core reset at session init restores nominal clocks
os.environ.setdefault("NEURON_RT_RESET_CORES", "1")

import concourse.bass as bass
import concourse.mybir as mybir
import concourse.tile as tile
from concourse import bacc
from concourse.bass_utils import run_bass_kernel_spmd

dt = mybir.dt
bf16 = ml_dtypes.bfloat16

N_CORES = 8
N_NODES = 20000
IN_CH = 128
HID = 32
HEADS = 8
HC = HEADS * HID  # 256
OUT_CH = 64
NEG_SLOPE = 0.2

P = 128
N_TILES_TOTAL = 157  # ceil(20000/128)
TILES_PER_CORE = 20  # 8*20 = 160 >= 157
N_PAD = 160 * P      # 20480
LOC_NODES = TILES_PER_CORE * P  # 2560
HALF = TILES_PER_CORE // 2

ROW1 = 384  # Hbuf row (bf16): [H'(256) | ones(8) | asrc(8) | adst(8) | pad]
ROW2 = 128  # h2 row (bf16):   [hW2(64) | one | asrc2 | adst2 | pad]

AF = mybir.ActivationFunctionType
OP = mybir.AluOpType

LAST_RESULTS = None


def _prep_edges(edge_index):
    src = np.asarray(edge_index[0], dtype=np.int64)
    dst = np.asarray(edge_index[1], dtype=np.int64)
    loops = np.arange(N_NODES, dtype=np.int64)
    src = np.concatenate([src, loops])
    dst = np.concatenate([dst, loops])

    order = np.lexsort((src, dst))
    src, dst = src[order], dst[order]
    tile_of = dst // P
    core_of = np.minimum(tile_of // TILES_PER_CORE, N_CORES - 1)

    per = [[None] * TILES_PER_CORE for _ in range(N_CORES)]
    for c in range(N_CORES):
        mc = core_of == c
        sc, tc_, dc = src[mc], tile_of[mc], dst[mc]
        for j in range(TILES_PER_CORE):
            gt = c * TILES_PER_CORE + j
            mt = tc_ == gt
            per[c][j] = (sc[mt], dc[mt] - gt * P)

    G = [max(max(1, -(-len(per[c][j][0]) // P)) for c in range(N_CORES))
         for j in range(TILES_PER_CORE)]

    # phase-D edge lists split by src half (h2allA/h2allB row spaces):
    # src g -> core c2=g//2560, local l=g%2560, half=l//1280,
    # row in h2all{A,B} = c2*1280 + l%1280
    perD = [[None] * TILES_PER_CORE for _ in range(N_CORES)]
    for c in range(N_CORES):
        for j in range(TILES_PER_CORE):
            s, dl = per[c][j]
            c2 = s // LOC_NODES
            l = s % LOC_NODES
            hf = l // (HALF * P)
            row = c2 * (HALF * P) + l % (HALF * P)
            perD[c][j] = ((row[hf == 0], dl[hf == 0]),
                          (row[hf == 1], dl[hf == 1]))
    GD = [[max(max(1, -(-len(perD[c][j][h][0]) // P)) for c in range(N_CORES))
           for j in range(TILES_PER_CORE)] for h in range(2)]

    def pack(lists, Gtab):
        """lists[j] = (idx_array, dl_array); returns idx16, dlc, dlr."""
        idx_cols, dl_lin = [], []
        for j in range(TILES_PER_CORE):
            s, dl = lists[j]
            n_pad = Gtab[j] * P
            sp = np.zeros(n_pad, dtype=np.int64)
            sp[: len(s)] = s
            dlp = np.full(n_pad, 200, dtype=np.int64)
            dlp[: len(dl)] = dl
            # dma_gather: idx k -> partition k%128, chunk k//128
            idx16 = sp.astype(np.int16).reshape(n_pad // 16, 16).T
            idx_cols.append(np.tile(idx16, (8, 1)))
            dl_lin.append(dlp)
        dl_lin = np.concatenate(dl_lin)
        ncht = len(dl_lin) // P
        # dlr rides a 128-partition broadcast DMA -> int8 to halve the bytes
        # (pad 200 wraps to -56: still never equal to a partition index)
        return (np.ascontiguousarray(np.concatenate(idx_cols, axis=1)),
                np.ascontiguousarray(dl_lin.reshape(ncht, P).T).astype(bf16),
                np.ascontiguousarray(dl_lin[None, :]).astype(np.int8))

    # Hbuf rows are stored partition-major (row = (g%128)*160 + g//128) so
    # phase A's staged writes are one contiguous descriptor per partition;
    # the gather just uses remapped indices
    perH = [[((s % P) * 160 + s // P, dl) for (s, dl) in per[c]]
            for c in range(N_CORES)]
    meta = []
    for c in range(N_CORES):
        idx, dlc, dlr = pack(perH[c], G)
        i0, c0, r0 = pack([perD[c][j][0] for j in range(TILES_PER_CORE)], GD[0])
        i1, c1, r1 = pack([perD[c][j][1] for j in range(TILES_PER_CORE)], GD[1])
        meta.append({"idx": idx, "dlc": dlc, "dlr": dlr,
                     "idx0": i0, "dlc0": c0, "dlr0": r0,
                     "idx1": i1, "dlc1": c1, "dlr1": r1})
    return G, GD, meta


def _build_program(G, GD):
    NCH = sum(G)
    NCH0 = sum(GD[0])
    NCH1 = sum(GD[1])
    nc = bacc.Bacc(None, target_bir_lowering=False, debug=False,
                   num_swdge_queues=4)

    xT = nc.dram_tensor("xT", [P, N_PAD], dt.bfloat16, kind="ExternalInput")
    xTloc = nc.dram_tensor("xTloc", [P, LOC_NODES], dt.bfloat16, kind="ExternalInput")
    W1a = nc.dram_tensor("W1a", [P, HC + 32], dt.bfloat16, kind="ExternalInput")
    W2a = nc.dram_tensor("W2a", [HC, OUT_CH + 2], dt.bfloat16, kind="ExternalInput")
    csT = nc.dram_tensor("cs", [P, OUT_CH + 2], dt.float32, kind="ExternalInput")
    b1r = nc.dram_tensor("b1r", [P, HC], dt.float32, kind="ExternalInput")
    b2r = nc.dram_tensor("b2r", [P, OUT_CH], dt.float32, kind="ExternalInput")
    idxT = nc.dram_tensor("idx", [P, 8 * NCH], dt.int16, kind="ExternalInput")
    dlcT = nc.dram_tensor("dlc", [P, NCH], dt.bfloat16, kind="ExternalInput")
    idx0T = nc.dram_tensor("idx0", [P, 8 * NCH0], dt.int16, kind="ExternalInput")
    dlc0T = nc.dram_tensor("dlc0", [P, NCH0], dt.bfloat16, kind="ExternalInput")
    idx1T = nc.dram_tensor("idx1", [P, 8 * NCH1], dt.int16, kind="ExternalInput")
    dlc1T = nc.dram_tensor("dlc1", [P, NCH1], dt.bfloat16, kind="ExternalInput")
    outT = nc.dram_tensor("out", [LOC_NODES, OUT_CH], dt.float32, kind="ExternalOutput")

    GRP = 8
    Gmax = max(G)
    Gmax2 = max(max(GD[0]), max(GD[1]))
    # gathers ride SWDGE queues 1-3: queue k's descgen runs on Q7 core pair
    # k and (unlike queue 0) retires on the Pool engine in ~100ns, so three
    # pairs generate descriptors concurrently while the engine stays free
    qstate = [0]

    def qn():
        qstate[0] = qstate[0] % 3 + 1
        return qstate[0]

    with tile.TileContext(nc) as tc:
        with (
            tc.tile_pool(name="const", bufs=1) as cp,
            tc.tile_pool(name="stage", bufs=2) as stp,
            tc.tile_pool(name="stageA", bufs=3) as stpA,
            tc.tile_pool(name="edgeg", bufs=5) as epg,
            tc.tile_pool(name="edges", bufs=2) as eps,
            tc.tile_pool(name="psA", bufs=1, space="PSUM") as ppA,
            tc.tile_pool(name="psB", bufs=2, space="PSUM") as ppB,
            tc.tile_pool(name="psC", bufs=1, space="PSUM") as ppC,
            tc.tile_pool(name="psD", bufs=1, space="PSUM") as ppD,
            tc.tile_pool(name="dram", bufs=1, space="DRAM") as dp,
        ):
            # ---- constants ----
            w1a_sb = cp.tile([P, HC + 32], dt.bfloat16)
            nc.sync.dma_start(w1a_sb[:], W1a[:, :])
            w2a_sb = cp.tile([P, 2, OUT_CH + 2], dt.bfloat16)
            nc.sync.dma_start(w2a_sb[:],
                              W2a[:, :].rearrange("(k p) n -> p k n", p=P))
            cs_sb = cp.tile([P, OUT_CH + 2], dt.float32)
            nc.sync.dma_start(cs_sb[:], csT[:, :])
            b1_sb = cp.tile([P, HC], dt.float32)
            nc.sync.dma_start(b1_sb[:], b1r[:, :])
            b2_sb = cp.tile([P, OUT_CH], dt.float32)
            nc.sync.dma_start(b2_sb[:], b2r[:, :])
            idx_sb = cp.tile([P, 8 * NCH], dt.int16)
            nc.sync.dma_start(idx_sb[:], idxT[:, :])
            dlc_sb = cp.tile([P, -(-NCH // P) * P], dt.bfloat16)
            nc.sync.dma_start(dlc_sb[:, 0:NCH], dlcT[:, :])
            idx0_sb = cp.tile([P, 8 * NCH0], dt.int16)
            nc.sync.dma_start(idx0_sb[:], idx0T[:, :])
            dlc0_sb = cp.tile([P, -(-NCH0 // P) * P], dt.bfloat16)
            nc.sync.dma_start(dlc0_sb[:, 0:NCH0], dlc0T[:, :])
            idx1_sb = cp.tile([P, 8 * NCH1], dt.int16)
            nc.sync.dma_start(idx1_sb[:], idx1T[:, :])
            dlc1_sb = cp.tile([P, -(-NCH1 // P) * P], dt.bfloat16)
            nc.sync.dma_start(dlc1_sb[:, 0:NCH1], dlc1T[:, :])

            iot_row = cp.tile([P, P], dt.float32)
            nc.gpsimd.iota(iot_row[:], pattern=[[1, P]], base=0,
                           channel_multiplier=0,
                           allow_small_or_imprecise_dtypes=True)
            iot_col = cp.tile([P, 1], dt.float32)
            nc.gpsimd.iota(iot_col[:], pattern=[[0, 1]], base=0,
                           channel_multiplier=1,
                           allow_small_or_imprecise_dtypes=True)
            iot_row_bf = cp.tile([P, P], dt.bfloat16)
            nc.vector.tensor_copy(iot_row_bf[:], iot_row[:])
            iot_col_bf = cp.tile([P, 1], dt.bfloat16)
            nc.vector.tensor_copy(iot_col_bf[:], iot_col[:])
            ident = cp.tile([P, P], dt.bfloat16)
            nc.vector.tensor_scalar(ident[:], iot_row[:], iot_col[:], None,
                                    op0=OP.is_equal)
            zeros_sb = cp.tile([P, HC], dt.float32)
            nc.vector.memset(zeros_sb[:], 0.0)
            ones_sb = cp.tile([P, P], dt.bfloat16)
            nc.vector.memset(ones_sb[:], 1.0)

            Hbuf = dp.tile([N_PAD, ROW1], dt.bfloat16)
            h2rows = dp.tile([LOC_NODES, ROW2], dt.bfloat16)
            h2allA = dp.tile([N_CORES * HALF * P, ROW2], dt.bfloat16,
                             addr_space="Shared")
            h2allB = dp.tile([N_CORES * HALF * P, ROW2], dt.bfloat16,
                             addr_space="Shared")

            # ---- phase A: GEMM1 for all node tiles -> Hbuf rows ----
            # Hbuf row: [H'(256) | zeros(8) | asrc(8) | adst(8) | pad];
            # rows partition-major so each partition's group-write is one
            # contiguous descriptor (cols 288:384 are never-read garbage)
            hview = Hbuf[:, :].rearrange("(p m) r -> p m r", m=160)
            for grp in range((N_TILES_TOTAL + GRP - 1) // GRP):
                n_in = min(GRP, N_TILES_TOTAL - grp * GRP)
                stg = stpA.tile([P, GRP, ROW1], dt.bfloat16, tag="stgA")
                xch = stpA.tile([P, GRP * P], dt.bfloat16, tag="xch")
                nc.sync.dma_start(xch[:, 0:n_in * P],
                                  xT[:, grp * GRP * P:(grp * GRP + n_in) * P])
                for k in range(n_in):
                    pool = ppA if k % 2 == 0 else ppB
                    tag = "psA" if k % 2 == 0 else "ps_o"
                    psA = pool.tile([P, HC + 32], dt.float32, tag=tag)
                    nc.tensor.matmul(psA[:],
                                     xch[:, k * P:(k + 1) * P],
                                     w1a_sb[:], start=True, stop=True)
                    if k % 2 == 0:
                        nc.scalar.copy(stg[:, k, 0:HC + 32], psA[:])
                    else:
                        nc.vector.tensor_copy(stg[:, k, 0:HC + 32], psA[:])
                nc.sync.dma_start(
                    hview[:, grp * GRP:grp * GRP + n_in, :],
                    stg[:, 0:n_in, :])

            # ---- local [asrc|adst] for this core's dst tiles ----
            xl_sb = cp.tile([P, LOC_NODES], dt.bfloat16)
            nc.sync.dma_start(xl_sb[:], xTloc[:, :])
            loc_a = cp.tile([P, TILES_PER_CORE, 16], dt.bfloat16)
            for jh in range(TILES_PER_CORE):
                psL = ppA.tile([P, HC + 32], dt.float32, tag="psA")
                nc.tensor.matmul(psL[:, 0:16],
                                 xl_sb[:, jh * P:(jh + 1) * P],
                                 w1a_sb[:, HC + 8:HC + 24], start=True, stop=True)
                nc.vector.tensor_copy(loc_a[:, jh:jh + 1, :],
                                      psL[:, 0:16].unsqueeze(1))

            loc_adst2 = cp.tile([P, TILES_PER_CORE, 1], dt.bfloat16)
            coff = [0]
            for j in range(1, TILES_PER_CORE + 1):
                coff.append(coff[-1] + G[j - 1])

            def build_onehots(Gj, co, dlc_t):
                """s01[p,g,m] = (dl[p+128g]==m); t01[p,g,m] = (dl[m+128g]==p)."""
                # partition-broadcast of each chunk's dst-local vector via a
                # PE transpose of the broadcast dlc column (no DMA: a
                # broadcast DMA would wait on a cumulative HWDGE-lane sem
                # that entangles it with per-tile output writes, ~19us)
                # t01's is_equal reads ps_dlb from PSUM: the PSUM read port
                # is NOT the POOL-shared SBUF port, so the op is immune to
                # Q7 descgen port starvation (SBUF-sourced 4x is_equal
                # degrades 10-24x while gather descriptors are generated)
                t01 = eps.tile([P, Gj, P], dt.bfloat16, tag="t01",
                               padded_shape=[P, Gmax, P])
                for g0 in range(0, Gj, 8):
                    g1 = min(g0 + 8, Gj)
                    ps_dlb = ppD.tile([P, 8, P], dt.bfloat16, tag="ps_dlb")
                    for g in range(g0, g1):
                        nc.tensor.transpose(
                            ps_dlb[:, g - g0, :],
                            dlc_t[:, co + g:co + g + 1].to_broadcast([P, P]),
                            ident[:])
                    nc.vector.tensor_scalar(t01[:, g0:g1, :],
                                            ps_dlb[:, 0:g1 - g0, :],
                                            iot_col[:], None, op0=OP.is_equal)
                s01 = eps.tile([P, Gj, P], dt.bfloat16, tag="s01",
                               padded_shape=[P, Gmax, P])
                nc.vector.tensor_tensor(
                    s01[:],
                    iot_row_bf[:].unsqueeze(1).broadcast_to([P, Gj, P]),
                    dlc_t[:, co:co + Gj].unsqueeze(2)
                    .broadcast_to([P, Gj, P]),
                    op=OP.is_equal)
                return s01, t01

            # ---- phase B: layer-1 edges ----
            for j in range(TILES_PER_CORE):
                Gj = G[j]
                ni = Gj * P
                hg = epg.tile([P, Gj, ROW1], dt.bfloat16, tag="hg",
                              padded_shape=[P, Gmax, ROW1])
                nc.gpsimd.dma_gather(hg[:, 0:Gj, :], Hbuf[:, :],
                                     idx_sb[:, 8 * coff[j]:8 * (coff[j] + Gj)],
                                     num_idxs=ni, num_idxs_reg=ni,
                                     elem_size=ROW1, single_packet=False,
                                     queue_num=qn())
                s01, t01 = build_onehots(Gj, coff[j], dlc_sb)

                ps_ad = ppB.tile([P, Gmax, 8], dt.float32, tag="ps_ad")
                for g in range(Gj):
                    nc.tensor.matmul(ps_ad[:, g, :], t01[:, g, :],
                                     loc_a[:, j, 8:16], start=True, stop=True)
                er = stp.tile([P, Gj, 8], dt.float32, tag="er",
                              padded_shape=[P, Gmax, 8])
                nc.vector.tensor_tensor(er[:], hg[:, 0:Gj, HC + 8:HC + 16],
                                        ps_ad[:, 0:Gj, :], op=OP.add)
                er2 = stp.tile([P, Gj, 8], dt.float32, tag="er2",
                               padded_shape=[P, Gmax, 8])
                nc.vector.scalar_tensor_tensor(er2[:], er[:], NEG_SLOPE, er[:],
                                               op0=OP.mult, op1=OP.max)
                # alpha = exp(er2) written by ACT straight into the
                # denominator columns of mg; the multiply then only covers
                # the 256 H' columns (vs 384 before)
                alp = stp.tile([P, Gj, 8], dt.bfloat16, tag="alp",
                               padded_shape=[P, Gmax, 8])
                nc.scalar.activation(alp[:], er2[:], AF.Exp)
                mg = eps.tile([P, Gj, HC], dt.bfloat16, tag="mg",
                              padded_shape=[P, Gmax, HC])
                nc.vector.tensor_tensor(
                    mg[:, 0:Gj, :].rearrange("p g (c h) -> p g c h",
                                             h=HEADS),
                    hg[:, 0:Gj, 0:HC].rearrange("p g (c h) -> p g c h",
                                                h=HEADS),
                    alp[:].unsqueeze(2).broadcast_to([P, Gj, HID, 8]),
                    op=OP.mult)

                # two accumulation chains into disjoint psum columns: the
                # softmax denominator comes from alpha fed to the PE
                # directly (a strided vector copy into an mg tail costs
                # ~6.4us on the slow-uop path)
                ps_of = ppB.tile([P, HC + 32], dt.float32, tag="ps_o")
                ps_o = ps_of[:, 0:HC + 8]
                for g in range(Gj):
                    nc.tensor.matmul(ps_of[:, 0:HC], s01[:, g, :],
                                     mg[:, g, :],
                                     start=(g == 0), stop=(g == Gj - 1))
                for g in range(Gj):
                    nc.tensor.matmul(ps_of[:, HC:HC + 8], s01[:, g, :],
                                     alp[:, g, :],
                                     start=(g == 0), stop=(g == Gj - 1))

                den = stp.tile([P, 8], dt.float32, tag="den")
                nc.vector.tensor_scalar(den[:], ps_o[:, HC:HC + 8], 1e-16,
                                        None, op0=OP.add)
                rec = stp.tile([P, 8], dt.float32, tag="rec")
                nc.vector.reciprocal(rec[:], den[:])
                o1 = stp.tile([P, HC], dt.float32, tag="o1")
                nc.vector.tensor_tensor(
                    o1[:].rearrange("p (c h) -> p c h", h=HEADS),
                    ps_o[:, 0:HC].rearrange("p (c h) -> p c h", h=HEADS),
                    rec[:].unsqueeze(1).broadcast_to([P, HID, 8]),
                    op=OP.mult)
                o1b = stp.tile([P, HC], dt.float32, tag="o1b")
                nc.vector.tensor_tensor(o1b[:], o1[:], b1_sb[:], op=OP.add)
                # ELU+1 (the -1 is folded into the cs correction after GEMM2)
                # min via tensor_tensor against zeros: tensor_scalar MIN runs
                # on a pathologically slow uop (~5.9us vs ~0.3us)
                en = stp.tile([P, HC], dt.float32, tag="en")
                nc.vector.tensor_tensor(en[:], o1b[:], zeros_sb[:], op=OP.min)
                ex = stp.tile([P, HC], dt.float32, tag="ex")
                nc.scalar.activation(ex[:], en[:], AF.Exp)
                h2p = stp.tile([P, HC], dt.bfloat16, tag="h2p")
                nc.vector.scalar_tensor_tensor(h2p[:], o1b[:], 0.0, ex[:],
                                               op0=OP.max, op1=OP.add)

                h2T = stp.tile([P, 2, P], dt.bfloat16, tag="h2T")
                for k in range(2):
                    pst = ppC.tile([P, P], dt.bfloat16, tag="pst")
                    nc.tensor.transpose(pst[:], h2p[:, k * P:(k + 1) * P],
                                        ident[:])
                    nc.vector.tensor_copy(h2T[:, k, :], pst[:])
                ps2 = ppC.tile([P, OUT_CH + 2], dt.float32, tag="ps2")
                for k in range(2):
                    nc.tensor.matmul(ps2[:], h2T[:, k, :], w2a_sb[:, k, :],
                                     start=(k == 0), stop=(k == 1))
                # row2: [hW2(64) | one | asrc2 | adst2]; (h2p-1)@W2a = ps2-cs
                row2 = stp.tile([P, ROW2], dt.bfloat16, tag="row2")
                nc.vector.tensor_tensor(row2[:, 0:OUT_CH], ps2[:, 0:OUT_CH],
                                        cs_sb[:, 0:OUT_CH], op=OP.subtract)
                nc.vector.tensor_tensor(row2[:, OUT_CH + 1:OUT_CH + 3],
                                        ps2[:, OUT_CH:OUT_CH + 2],
                                        cs_sb[:, OUT_CH:OUT_CH + 2],
                                        op=OP.subtract)
                nc.vector.tensor_copy(loc_adst2[:, j, :],
                                      row2[:, OUT_CH + 2:OUT_CH + 3])
                nc.sync.dma_start(
                    h2rows[:, :].rearrange("(t p) r -> p t r", p=P)
                    [:, j, 0:OUT_CH + 3],
                    row2[:, 0:OUT_CH + 3])

                # first-half allgather as soon as tiles 0..HALF-1 are done
                if j == HALF - 1:
                    nc.gpsimd.collective_compute(
                        "AllGather", OP.bypass,
                        replica_groups=[list(range(N_CORES))],
                        ins=[h2rows[0:HALF * P, :].opt()],
                        outs=[h2allA[:, :].opt()])

            # ---- phase C: second-half allgather ----
            nc.gpsimd.collective_compute(
                "AllGather", OP.bypass,
                replica_groups=[list(range(N_CORES))],
                ins=[h2rows[HALF * P:LOC_NODES, :].opt()],
                outs=[h2allB[:, :].opt()])

            # ---- phase D: layer-2 edges, split by src half ----
            # half-0 gathers/compute depend only on h2allA (ready during
            # phase B), so they fill the AG2 bubble; partial sums park in
            # SBUF until half-1.
            coff0 = [0]
            for j in range(1, TILES_PER_CORE + 1):
                coff0.append(coff0[-1] + GD[0][j - 1])
            coff1 = [0]
            for j in range(1, TILES_PER_CORE + 1):
                coff1.append(coff1[-1] + GD[1][j - 1])
            part3 = cp.tile([P, TILES_PER_CORE, OUT_CH + 1], dt.float32)

            def phase_d_half(j, Gj, co, h2src, idx_t, dlc_t):
                ni = Gj * P
                hg2 = epg.tile([P, Gj, ROW2], dt.bfloat16, tag="hg2",
                               padded_shape=[P, Gmax2, ROW2])
                nc.gpsimd.dma_gather(hg2[:, 0:Gj, :], h2src[:, :],
                                     idx_t[:, 8 * co:8 * (co + Gj)],
                                     num_idxs=ni, num_idxs_reg=ni,
                                     elem_size=ROW2, single_packet=False,
                                     queue_num=qn())
                s01, t01 = build_onehots(Gj, co, dlc_t)

                ps_a2f = ppB.tile([P, Gmax, 8], dt.float32, tag="ps_ad")
                ps_a2 = ps_a2f[:, :, 0:1]
                for g in range(Gj):
                    nc.tensor.matmul(ps_a2[:, g, :], t01[:, g, :],
                                     loc_adst2[:, j, :], start=True, stop=True)
                e2 = stp.tile([P, Gj, 1], dt.float32, tag="e2",
                              padded_shape=[P, Gmax2, 1])
                nc.vector.tensor_tensor(e2[:],
                                        hg2[:, 0:Gj, OUT_CH + 1:OUT_CH + 2],
                                        ps_a2[:, 0:Gj, :], op=OP.add)
                e2b = stp.tile([P, Gj, 1], dt.float32, tag="e2b",
                               padded_shape=[P, Gmax2, 1])
                nc.vector.scalar_tensor_tensor(e2b[:], e2[:], NEG_SLOPE, e2[:],
                                               op0=OP.mult, op1=OP.max)
                # alpha2 into the denominator column (64) directly; multiply
                # covers only the 64 hW2 columns (vs 128 before)
                al2 = stp.tile([P, Gj, 1], dt.bfloat16, tag="al2",
                               padded_shape=[P, Gmax2, 1])
                nc.scalar.activation(al2[:], e2b[:], AF.Exp)
                mg2 = eps.tile([P, Gj, OUT_CH], dt.bfloat16, tag="mg2",
                               padded_shape=[P, Gmax2, OUT_CH])
                nc.vector.tensor_tensor(
                    mg2[:, 0:Gj, :], hg2[:, 0:Gj, 0:OUT_CH],
                    al2[:].broadcast_to([P, Gj, OUT_CH]),
                    op=OP.mult)

                ps3f = ppC.tile([P, OUT_CH + 2], dt.float32, tag="ps2")
                ps3 = ps3f[:, 0:OUT_CH + 1]
                for g in range(Gj):
                    nc.tensor.matmul(ps3f[:, 0:OUT_CH], s01[:, g, :],
                                     mg2[:, g, :],
                                     start=(g == 0), stop=(g == Gj - 1))
                for g in range(Gj):
                    nc.tensor.matmul(ps3f[:, OUT_CH:OUT_CH + 1],
                                     s01[:, g, :], al2[:, g, :],
                                     start=(g == 0), stop=(g == Gj - 1))
                return ps3

            for j in range(TILES_PER_CORE):
                ps3 = phase_d_half(j, GD[0][j], coff0[j], h2allA,
                                   idx0_sb, dlc0_sb)
                nc.vector.tensor_copy(part3[:, j, :], ps3[:])

            for j in range(TILES_PER_CORE):
                ps3 = phase_d_half(j, GD[1][j], coff1[j], h2allB,
                                   idx1_sb, dlc1_sb)
                o3s = stp.tile([P, OUT_CH + 1], dt.float32, tag="o3s")
                nc.vector.tensor_tensor(o3s[:], ps3[:], part3[:, j, :],
                                        op=OP.add)
                den2 = stp.tile([P, 1], dt.float32, tag="den2")
                nc.vector.tensor_scalar(den2[:], o3s[:, OUT_CH:OUT_CH + 1],
                                        1e-16, None, op0=OP.add)
                rec2 = stp.tile([P, 1], dt.float32, tag="rec2")
                nc.vector.reciprocal(rec2[:], den2[:])
                o2 = stp.tile([P, OUT_CH], dt.float32, tag="o2")
                nc.vector.tensor_tensor(o2[:], o3s[:, 0:OUT_CH],
                                        rec2[:].broadcast_to([P, OUT_CH]),
                                        op=OP.mult)
                o2b = stp.tile([P, OUT_CH], dt.float32, tag="o2b")
                nc.vector.tensor_tensor(o2b[:], o2[:], b2_sb[:], op=OP.add)
                nc.sync.dma_start(
                    outT[:, :].rearrange("(t p) r -> p t r", p=P)[:, j, :],
                    o2b[:])

    nc.compile()
    return nc


def kernel(x, edge_index, W1, a_src1, a_dst1, b1, W2, a_src2, a_dst2, b2,
           _trace=False, _tmpdir=None):
    x = np.asarray(x, dtype=np.float32)
    W1 = np.asarray(W1, dtype=np.float32)
    a_src1 = np.asarray(a_src1, dtype=np.float32)
    a_dst1 = np.asarray(a_dst1, dtype=np.float32)
    b1 = np.asarray(b1, dtype=np.float32)
    W2 = np.asarray(W2, dtype=np.float32)
    a_src2 = np.asarray(a_src2, dtype=np.float32)
    a_dst2 = np.asarray(a_dst2, dtype=np.float32)
    b2 = np.asarray(b2, dtype=np.float32)

    G, GD, meta = _prep_edges(edge_index)

    A1 = np.zeros((HC, 16), np.float32)
    for h in range(HEADS):
        A1[h * HID:(h + 1) * HID, h] = a_src1[h]
        A1[h * HID:(h + 1) * HID, 8 + h] = a_dst1[h]
    # W1' columns head-interleaved: col c*8+h = W1 col h*32+c
    W1i = np.ascontiguousarray(
        W1.reshape(IN_CH, HEADS, HID).transpose(0, 2, 1).reshape(IN_CH, HC))
    W1a = np.ascontiguousarray(
        np.concatenate([W1i, np.zeros((IN_CH, 8), np.float32), W1 @ A1,
                        np.zeros((IN_CH, 8), np.float32)],
                       axis=1)).astype(bf16)
    # W2 rows permuted to match interleaved h2 (row c*8+h = W2 row h*32+c)
    W2i = np.ascontiguousarray(
        W2.reshape(HEADS, HID, OUT_CH).transpose(1, 0, 2).reshape(HC, OUT_CH))
    a_src2i = a_src2  # [1, 64] acts on output cols, not affected
    W2af = np.concatenate([W2i, W2i @ a_src2.T, W2i @ a_dst2.T], axis=1)
    W2a = np.ascontiguousarray(W2af).astype(bf16)
    # colsum correction (h2 = h2p - 1): subtract colsum(W2a) after GEMM2
    cs = np.ascontiguousarray(
        np.tile(W2af.astype(np.float32).sum(axis=0)[None, :], (P, 1)))
    b1i = b1.reshape(HEADS, HID).T.reshape(HC)

    xT = np.zeros((P, N_PAD), np.float32)
    xT[:, :N_NODES] = x.T
    xT = xT.astype(bf16)
    b1r = np.ascontiguousarray(np.tile(b1i[None, :], (P, 1)).astype(np.float32))
    b2r = np.ascontiguousarray(np.tile(b2[None, :], (P, 1)).astype(np.float32))

    nc = _build_program(G, GD)

    in_maps = []
    for c in range(N_CORES):
        in_maps.append({
            "xT": xT,
            "xTloc": np.ascontiguousarray(
                xT[:, c * LOC_NODES:(c + 1) * LOC_NODES]),
            "W1a": W1a, "W2a": W2a, "cs": cs, "b1r": b1r, "b2r": b2r,
            "idx": meta[c]["idx"], "dlc": meta[c]["dlc"],
            "idx0": meta[c]["idx0"], "dlc0": meta[c]["dlc0"],
            "idx1": meta[c]["idx1"], "dlc1": meta[c]["dlc1"],
        })

    res = run_bass_kernel_spmd(nc, in_maps, core_ids=list(range(N_CORES)),
                               trace=_trace, tmpdir=_tmpdir)
    global LAST_RESULTS
    LAST_RESULTS = res
    out = np.concatenate([res.results[c]["out"] for c in range(N_CORES)], axis=0)
    return out[:N_NODES]

